# revision 27
# baseline (speedup 1.0000x reference)
"""CRF negative-log-likelihood kernel for Trainium2 (8 NeuronCores, batch-sharded).

Algorithm:
  - Launch 1 (vocab-sharded): t2 = embedding @ fc_w in bf16. Host pre-transposes
    the embedding shard so the kernel is just convert-to-bf16 + 50 matmuls
    (lhsT = embT chunk, rhs = fc_w), no PE transposes. Output t2 is bf16
    (32B rows) to halve gather traffic.
  - Launch 2 (batch-sharded, 8 rows/core, bf16 compute): merged indirect-DMA
    gathers of t2 rows (8 calls, 4096 descriptors each), bf16 PE-block
    transposes into class-on-partition layout, numerator via one-hot matmul +
    multiply-reduce, and a segmented forward/backward scan (L=16 steps, S=256
    segments on the free dim) in linear space with the two scan chains
    interleaved so vector muls hide behind the other chain's matmuls.
  - Host (float64, vectorized): rank-1 junction chain across segments, exact
    partial segment for each row's ragged tail, final scalar assembly.
"""
import sys
sys.path.insert(0, "/opt/trn_rl_repo")
import numpy as np
import ml_dtypes
from contextlib import ExitStack

import concourse.bass as bass
import concourse.bacc as bacc_mod
import concourse.mybir as mybir
import concourse.tile as tile
from concourse.masks import make_identity
from concourse.bass_utils import run_bass_kernel_spmd

F32 = mybir.dt.float32
BF16 = mybir.dt.bfloat16
I32 = mybir.dt.int32
NPBF = ml_dtypes.bfloat16

V, E, C = 50257, 128, 16
B, T = 64, 4096
L, S = 8, 512
VPAD = 51200
VSH = VPAD // 8
BL = 8
NCHUNK = 8
CHW = T // NCHUNK
NCORES = 8

LAST_EXEC_NS = {}
_TRACE = False
_CACHE = {}


def build_t2_kernel():
    nc = bacc_mod.Bacc()
    # embT_s: host-pretransposed shard, (E, VSH) f32
    embT_s = nc.dram_tensor("embT_s", [E, VSH], F32, kind="ExternalInput")
    fc_w = nc.dram_tensor("fc_w", [E, C], F32, kind="ExternalInput")
    # t2 shard in (partition, chunk*C) layout: row v = i*128+p lives at
    # [p, i*C:(i+1)*C]; host permutes back. Keeps the out-DMA contiguous
    # per partition (1600B runs instead of 32B runs).
    ntile = VSH // 128          # 50 chunks of 128 vocab rows
    t2_s = nc.dram_tensor("t2_s", [128, ntile * C], BF16, kind="ExternalOutput")

    NGRP = 10                   # DMA/convert granularity: 5 chunks per group
    GW = VSH // NGRP            # 640 columns per group
    with ExitStack() as ctx:
        tc = ctx.enter_context(tile.TileContext(nc))
        singles = ctx.enter_context(tc.tile_pool(name="singles", bufs=1))
        psum = ctx.enter_context(tc.tile_pool(name="psum", bufs=4, space="PSUM"))

        fcw_f32 = singles.tile([E, C], F32)
        nc.scalar.dma_start(out=fcw_f32[:], in_=fc_w[:])
        fcw_bf = singles.tile([E, C], BF16)
        nc.vector.tensor_copy(fcw_bf[:], fcw_f32[:])

        EMBT = singles.tile([128, VSH], F32)
        EMBTb = singles.tile([128, VSH], BF16)
        T2 = singles.tile([128, ntile * C], BF16)
        for g in range(NGRP):
            eng = nc.sync if g % 2 == 0 else nc.scalar
            eng.dma_start(out=EMBT[:, g * GW:(g + 1) * GW],
                          in_=embT_s[:, g * GW:(g + 1) * GW])
            if g % 2 == 0:
                nc.vector.tensor_copy(EMBTb[:, g * GW:(g + 1) * GW],
                                      EMBT[:, g * GW:(g + 1) * GW])
            else:
                nc.scalar.copy(EMBTb[:, g * GW:(g + 1) * GW],
                               EMBT[:, g * GW:(g + 1) * GW])
        for i in range(ntile):
            ps2 = psum.tile([128, C], F32, tag="p2")
            nc.tensor.matmul(ps2[:], lhsT=EMBTb[:, i * 128:(i + 1) * 128],
                             rhs=fcw_bf[:], start=True, stop=True)
            if i % 2 == 0:
                nc.vector.tensor_copy(T2[:, i * C:(i + 1) * C], ps2[:])
            else:
                nc.scalar.copy(T2[:, i * C:(i + 1) * C], ps2[:])
        nc.sync.dma_start(out=t2_s[:], in_=T2[:])
    return nc


def _strided(base_ap, k, step, count):
    return bass.AP(tensor=base_ap.tensor, offset=base_ap.offset + k,
                   ap=[base_ap.ap[0], [step, count]])


def build_main_kernel():
    nc = bacc_mod.Bacc()
    x_t = nc.dram_tensor("x_t", [128, T // 128 * BL], I32, kind="ExternalInput")
    tags_f = nc.dram_tensor("tags_f", [BL, T], BF16, kind="ExternalInput")
    t2 = nc.dram_tensor("t2", [VPAD, C], BF16, kind="ExternalInput")
    blockP = nc.dram_tensor("blockP", [128, 128], BF16, kind="ExternalInput")
    blockPT = nc.dram_tensor("blockPT", [128, 128], BF16, kind="ExternalInput")
    bcast8 = nc.dram_tensor("bcast8", [BL, 128], BF16, kind="ExternalInput")
    iota_rep = nc.dram_tensor("iota_rep", [128, CHW], BF16, kind="ExternalInput")
    sadj = nc.dram_tensor("sadj", [128, 1], BF16, kind="ExternalInput")

    r_out = nc.dram_tensor("r_out", [128, S], BF16, kind="ExternalOutput")
    d_out = nc.dram_tensor("d_out", [128, S], BF16, kind="ExternalOutput")
    num_out = nc.dram_tensor("num_out", [1, 2 * NCHUNK], F32, kind="ExternalOutput")

    with ExitStack() as ctx:
        tc = ctx.enter_context(tile.TileContext(nc))
        singles = ctx.enter_context(tc.tile_pool(name="singles", bufs=1))
        big = ctx.enter_context(tc.tile_pool(name="big", bufs=1))
        scratch = ctx.enter_context(tc.tile_pool(name="scratch", bufs=3))
        psum = ctx.enter_context(tc.tile_pool(name="psum", bufs=2, space="PSUM"))
        psum2 = ctx.enter_context(tc.tile_pool(name="psum2", bufs=2, space="PSUM"))

        # input DMAs spread across queues for parallel issue
        xt_sb = singles.tile([128, T // 128 * BL], I32)
        nc.sync.dma_start(out=xt_sb[:], in_=x_t[:])
        tagsf_sb = singles.tile([BL, T], BF16)
        nc.scalar.dma_start(out=tagsf_sb[:], in_=tags_f[:])
        bcast8_sb = singles.tile([BL, 128], BF16)
        nc.scalar.dma_start(out=bcast8_sb[:], in_=bcast8[:])
        iotar_sb = singles.tile([128, CHW], BF16)
        nc.scalar.dma_start(out=iotar_sb[:], in_=iota_rep[:])
        blockP_sb = singles.tile([128, 128], BF16)
        nc.scalar.dma_start(out=blockP_sb[:], in_=blockP[:])
        blockPT_sb = singles.tile([128, 128], BF16)
        nc.scalar.dma_start(out=blockPT_sb[:], in_=blockPT[:])
        sadj_sb = singles.tile([128, 1], BF16)
        nc.scalar.dma_start(out=sadj_sb[:], in_=sadj[:])

        TM = big.tile([128, T], BF16)
        G = big.tile([128, T], BF16)
        EXPG = big.tile([128, T], BF16)
        W = big.tile([128, T], BF16)
        num_sb = singles.tile([1, 2 * NCHUNK], F32)

        nc.vector.memset(num_sb[:], 0.0)

        EXPGap = EXPG[:]
        Gap = G[:]

        # --- numerator W build first: only needs tags, runs during gather ---
        for c in range(NCHUNK):
            c0 = c * CHW
            psA = psum.tile([128, CHW], F32, tag="ps")
            nc.tensor.matmul(psA[:], lhsT=bcast8_sb[:],
                             rhs=tagsf_sb[:, c0:c0 + CHW], start=True, stop=True)
            nc.vector.tensor_tensor(out=W[:, c0:c0 + CHW], in0=psA[:],
                                    in1=iotar_sb[:], op=mybir.AluOpType.is_equal)

        # --- gather (token-major): one merged indirect DMA per chunk.
        # Offsets xt_sb[:, c*32:(c+1)*32] enumerate (partition, col) C-order;
        # each offset owns 16 contiguous bf16 of the dest view — identical
        # mapping to per-column calls but amortizes the ~1us SWDGE fixed cost.
        ncc = CHW // 16
        for c in range(NCHUNK):
            c0 = c * CHW
            nc.gpsimd.indirect_dma_start(
                out=TM[:, c0:c0 + CHW],
                out_offset=None,
                in_=t2[:],
                in_offset=bass.IndirectOffsetOnAxis(
                    ap=xt_sb[:, c * ncc:(c + 1) * ncc], axis=0),
            )

        # --- XBAR block-transpose TM -> G (2 chunks per call) + exp ---
        # out view (128, nb, 128): out[p, b, j] = TM[j, g0 + b*128 + p],
        # i.e. an independent transpose of each 128x128 block.
        # EXP writes EXPG in segment-major layout: token t lands at column
        # (t % L) * S + t // L, so scan step k reads the contiguous block
        # EXPG[:, k*S:(k+1)*S] (strided DVE reads cost ~2x).
        GRPW = 2 * CHW
        nbl = GRPW // 128
        for g in range(NCHUNK // 2):
            g0 = g * GRPW
            eng = nc.sync if g % 2 == 0 else nc.scalar
            eng.dma_start_transpose(
                out=bass.AP(tensor=Gap.tensor, offset=g0,
                            ap=[[Gap.ap[0][0], 128], [128, nbl], [1, 128]]),
                in_=TM[:, g0:g0 + GRPW])
            nc.scalar.activation(
                bass.AP(tensor=EXPGap.tensor, offset=g0 // L,
                        ap=[[EXPGap.ap[0][0], 128], [1, GRPW // L], [S, L]]),
                G[:, g0:g0 + GRPW],
                mybir.ActivationFunctionType.Exp)
        nc.vector.tensor_mul(EXPG[:, 0:1], EXPG[:, 0:1], sadj_sb[:])

        # --- numerator: em_tag only (transition terms done on host);
        # runs on the Pool engine so it never delays the DVE scan chain ---
        for c in range(NCHUNK):
            c0 = c * CHW
            scr2 = scratch.tile([128, CHW], BF16, tag="scr2")
            nc.gpsimd.tensor_mul(scr2[:], G[:, c0:c0 + CHW], W[:, c0:c0 + CHW])
            nc.gpsimd.reduce_sum(out=num_sb[:, c:c + 1], in_=scr2[:],
                                 axis=mybir.AxisListType.XYZWC)

        # --- scans: forward and backward chains interleaved ---
        r_sb = big.tile([128, S], BF16)
        nc.vector.memset(r_sb[:], 1.0)
        d_sb = big.tile([128, S], BF16)
        nc.vector.tensor_copy(d_sb[:], EXPG[:, (L - 1) * S:L * S])
        for i in range(L):
            kf = i               # forward step k = 0..L-1
            kb = L - 2 - i       # backward step k = L-2..0
            psR = psum2.tile([128, S], F32, tag="psR")
            nc.tensor.matmul(psR[:], lhsT=blockP_sb[:], rhs=r_sb[:],
                             start=True, stop=True)
            if kb >= 0:
                psD = psum2.tile([128, S], F32, tag="psD")
                nc.tensor.matmul(psD[:], lhsT=blockPT_sb[:], rhs=d_sb[:],
                                 start=True, stop=True)
            nc.vector.tensor_mul(r_sb[:], psR[:], EXPG[:, kf * S:(kf + 1) * S])
            if kb >= 0:
                nc.vector.tensor_mul(d_sb[:], psD[:], EXPG[:, kb * S:(kb + 1) * S])

        nc.sync.dma_start(out=r_out[:], in_=r_sb[:])
        nc.sync.dma_start(out=d_out[:], in_=d_sb[:])
        nc.sync.dma_start(out=num_out[:], in_=num_sb[:])
    return nc


def _host_prep(embedding, fc_w, fc_b, trans, start):
    P_eff64 = np.exp(trans.astype(np.float64) + fc_b[None, :].astype(np.float64))
    colsum = P_eff64.sum(0)
    start_adj = (np.exp(start.astype(np.float64) + fc_b) / colsum).astype(np.float32)
    trans_n = (trans + fc_b[None, :]).astype(np.float32)
    P_eff32 = P_eff64.astype(np.float32)

    eye8 = np.eye(BL, dtype=np.float32)
    return dict(
        P_eff=P_eff64,
        trans_n=trans_n.astype(np.float64),
        blockP=np.ascontiguousarray(np.kron(eye8, P_eff32)).astype(NPBF),
        blockPT=np.ascontiguousarray(np.kron(eye8, P_eff32.T.copy())).astype(NPBF),
        bcast8=np.ascontiguousarray(np.kron(eye8, np.ones((1, C), np.float32))).astype(NPBF),
        iota_rep=np.ascontiguousarray(
            np.tile(np.tile(np.arange(C, dtype=np.float32), BL)[:, None],
                    (1, CHW))).astype(NPBF),
        sadj=np.ascontiguousarray(np.tile(start_adj, BL)[:, None]).astype(NPBF),
    )


LAST_RESULTS = {}


def _run(nc, in_maps, label):
    res = run_bass_kernel_spmd(nc, in_maps, core_ids=list(range(NCORES)),
                               trace=_TRACE)
    if res.exec_time_ns is not None:
        LAST_EXEC_NS[label] = res.exec_time_ns
    LAST_RESULTS[label] = res
    return res.results


def kernel(x, tags, embedding, fc_w, fc_b, start_transitions, end_transitions,
           transitions):
    x = np.asarray(x, np.int32)
    tags = np.asarray(tags, np.int32)
    embedding = np.asarray(embedding, np.float32)
    fc_w = np.asarray(fc_w, np.float32)
    fc_b = np.asarray(fc_b, np.float32)
    trans = np.asarray(transitions, np.float32)
    start = np.asarray(start_transitions, np.float32)
    end = np.asarray(end_transitions, np.float32)

    prep = _host_prep(embedding, fc_w, fc_b, trans, start)

    if "t2" not in _CACHE:
        nc1 = build_t2_kernel()
        nc1.finalize()
        _CACHE["t2"] = nc1
    if "main" not in _CACHE:
        nc2 = build_main_kernel()
        nc2.finalize()
        _CACHE["main"] = nc2

    # ---- launch 1: t2 = emb_pad @ fc_w (bf16 out), vocab-sharded ----
    emb_pad_T = np.zeros((E, VPAD), np.float32)
    emb_pad_T[:, :V] = embedding.T
    in1 = [{"embT_s": np.ascontiguousarray(emb_pad_T[:, k * VSH:(k + 1) * VSH]),
            "fc_w": fc_w} for k in range(NCORES)]
    res1 = _run(_CACHE["t2"], in1, "t2")
    # t2_s comes back as (128, ntile*C): row v = i*128+p at [p, i*C:(i+1)*C]
    t2_full = np.concatenate(
        [np.asarray(res1[k]["t2_s"]).reshape(128, VSH // 128, C)
         .transpose(1, 0, 2).reshape(VSH, C) for k in range(NCORES)], axis=0)
    t2_full = np.ascontiguousarray(t2_full)          # (VPAD, C) bf16

    # ---- launch 2: main kernel, batch-sharded ----
    tags_m = np.where(x != 0, tags, C).astype(NPBF)
    in2 = []
    for k in range(NCORES):
        sl = slice(k * BL, (k + 1) * BL)
        xt = x[sl].reshape(BL, T // 128, 128).transpose(2, 1, 0) \
                  .reshape(128, T // 128 * BL)
        in2.append({
            "x_t": np.ascontiguousarray(xt),
            "tags_f": np.ascontiguousarray(tags_m[sl]),
            "t2": t2_full,
            "blockP": prep["blockP"], "blockPT": prep["blockPT"],
            "bcast8": prep["bcast8"],
            "iota_rep": prep["iota_rep"], "sadj": prep["sadj"],
        })
    res2 = _run(_CACHE["main"], in2, "main")

    # ---- host combine (float64, vectorized) ----
    lengths = (x != 0).sum(1)                        # (B,)
    start64 = start.astype(np.float64)
    end64 = end.astype(np.float64)
    fcb64 = fc_b.astype(np.float64)
    Pe = prep["P_eff"]                               # (C, C) float64
    t264 = t2_full.astype(np.float64)                # (VPAD, C)
    exp_end = np.exp(end64)

    em_total = sum(float(np.asarray(res2[k]["num_out"], np.float64).sum())
                   for k in range(NCORES))
    r = np.concatenate(
        [np.asarray(res2[k]["r_out"]).astype(np.float64).reshape(BL, C, S)
         for k in range(NCORES)], axis=0)            # (B, C, S)
    d = np.concatenate(
        [np.asarray(res2[k]["d_out"]).astype(np.float64).reshape(BL, C, S)
         for k in range(NCORES)], axis=0)            # (B, C, S)

    num = start64[tags[:, 0]] + fcb64[tags[:, 0]]
    num += end64[tags[np.arange(B), lengths - 1]]
    # transition terms (pure tags/params, no device data)
    maskf = (x[:, 1:] != 0).astype(np.float64)
    num += (prep["trans_n"][tags[:, :-1], tags[:, 1:]] * maskf).sum(axis=1)

    # full-segment junction chain: for s in 1..sstar-1:
    #   logZ += log(r[:,:,s-1] @ (Pe @ d[:,:,s])) - log(r[:,:,s].sum())
    sstar = (lengths - 1) // L                       # (B,)
    cs = np.einsum('cd,bds->bcs', Pe, d)             # (B, C, S)
    t1 = np.einsum('bcs,bcs->bs', r[:, :, :-1], cs[:, :, 1:])   # junction at s=1..S-1
    rs = r.sum(axis=1)                               # (B, S)
    s_idx = np.arange(1, S)[None, :]                 # (1, S-1)
    jmask = s_idx <= (sstar[:, None] - 1)            # (B, S-1)
    terms = np.where(jmask, np.log(t1) - np.log(rs[:, 1:]), 0.0)
    logZ = terms.sum(axis=1)                         # (B,)

    # ragged tail: exact alpha recursion from segment sstar-1's r
    alpha = r[np.arange(B), :, sstar - 1].copy()     # (B, C)
    tail_len = lengths - sstar * L                   # in [1, L]
    for t_off in range(L):
        active = t_off < tail_len                    # (B,)
        t_idx = np.minimum(sstar * L + t_off, T - 1)
        w = np.exp(t264[x[np.arange(B), t_idx]] + fcb64[None, :])   # (B, C)
        alpha_new = (alpha @ Pe) * w
        alpha = np.where(active[:, None], alpha_new, alpha)
    logZ += np.log(alpha @ exp_end)

    total = -(num - logZ).sum() - em_total
    return np.array(total, dtype=np.float32)


# revision 32
# speedup vs baseline: 1.1387x; 1.1387x over previous
"""CRF negative-log-likelihood kernel for Trainium2 (8 NeuronCores, batch-sharded).

Algorithm:
  - Launch 1 (vocab-sharded): t2 = embedding @ fc_w in bf16. Host pre-transposes
    the embedding shard so the kernel is just convert-to-bf16 + 50 matmuls
    (lhsT = embT chunk, rhs = fc_w), no PE transposes. Output t2 is bf16
    (32B rows) to halve gather traffic.
  - Launch 2 (batch-sharded, 8 rows/core, bf16 compute): merged indirect-DMA
    gathers of t2 rows (8 calls, 4096 descriptors each), bf16 PE-block
    transposes into class-on-partition layout, numerator via one-hot matmul +
    multiply-reduce, and a segmented forward/backward scan (L=16 steps, S=256
    segments on the free dim) in linear space with the two scan chains
    interleaved so vector muls hide behind the other chain's matmuls.
  - Host (float64, vectorized): rank-1 junction chain across segments, exact
    partial segment for each row's ragged tail, final scalar assembly.
"""
import sys
sys.path.insert(0, "/opt/trn_rl_repo")
import numpy as np
import ml_dtypes
from contextlib import ExitStack

import concourse.bass as bass
import concourse.bacc as bacc_mod
import concourse.mybir as mybir
import concourse.tile as tile
from concourse.masks import make_identity
from concourse.bass_utils import run_bass_kernel_spmd

F32 = mybir.dt.float32
BF16 = mybir.dt.bfloat16
I32 = mybir.dt.int32
NPBF = ml_dtypes.bfloat16

V, E, C = 50257, 128, 16
B, T = 64, 4096
L, S = 8, 512
VPAD = 51200
VSH = VPAD // 8
BL = 8
NCHUNK = 8
CHW = T // NCHUNK
NCORES = 8

LAST_EXEC_NS = {}
_TRACE = False
_CACHE = {}


def build_t2_kernel():
    nc = bacc_mod.Bacc()
    # embT_s: host-pretransposed shard, (E, VSH) f32
    embT_s = nc.dram_tensor("embT_s", [E, VSH], F32, kind="ExternalInput")
    fc_w = nc.dram_tensor("fc_w", [E, C], F32, kind="ExternalInput")
    # t2 shard TRANSPOSED: (C, VSH); host un-transposes. One stationary fcw,
    # wide matmuls (out 16 x 640), contiguous out-DMA.
    t2_s = nc.dram_tensor("t2_s", [C, VSH], BF16, kind="ExternalOutput")

    NGRP = 10                   # DMA/convert/matmul granularity
    GW = VSH // NGRP            # 640 columns per group
    with ExitStack() as ctx:
        tc = ctx.enter_context(tile.TileContext(nc))
        singles = ctx.enter_context(tc.tile_pool(name="singles", bufs=1))
        psum = ctx.enter_context(tc.tile_pool(name="psum", bufs=4, space="PSUM"))

        fcw_f32 = singles.tile([E, C], F32)
        nc.scalar.dma_start(out=fcw_f32[:], in_=fc_w[:])
        fcw_bf = singles.tile([E, C], BF16)
        nc.vector.tensor_copy(fcw_bf[:], fcw_f32[:])

        EMBT = singles.tile([128, VSH], F32)
        EMBTb = singles.tile([128, VSH], BF16)
        T2T = singles.tile([C, VSH], BF16)
        for g in range(NGRP):
            eng = nc.sync if g % 2 == 0 else nc.scalar
            eng.dma_start(out=EMBT[:, g * GW:(g + 1) * GW],
                          in_=embT_s[:, g * GW:(g + 1) * GW])
            if g % 2 == 0:
                nc.vector.tensor_copy(EMBTb[:, g * GW:(g + 1) * GW],
                                      EMBT[:, g * GW:(g + 1) * GW])
            else:
                nc.scalar.copy(EMBTb[:, g * GW:(g + 1) * GW],
                               EMBT[:, g * GW:(g + 1) * GW])
        MW = GW // 2            # 320 f32 fits a 2KB PSUM bank
        for m in range(2 * NGRP):
            ps2 = psum.tile([C, MW], F32, tag="p2")
            nc.tensor.matmul(ps2[:], lhsT=fcw_bf[:],
                             rhs=EMBTb[:, m * MW:(m + 1) * MW],
                             start=True, stop=True)
            if m % 2 == 0:
                nc.vector.tensor_copy(T2T[:, m * MW:(m + 1) * MW], ps2[:])
            else:
                nc.scalar.copy(T2T[:, m * MW:(m + 1) * MW], ps2[:])
        nc.sync.dma_start(out=t2_s[:], in_=T2T[:])
    return nc


def _strided(base_ap, k, step, count):
    return bass.AP(tensor=base_ap.tensor, offset=base_ap.offset + k,
                   ap=[base_ap.ap[0], [step, count]])


def build_main_kernel():
    nc = bacc_mod.Bacc()
    x_t = nc.dram_tensor("x_t", [128, T // 128 * BL], I32, kind="ExternalInput")
    tags_f = nc.dram_tensor("tags_f", [BL, T], BF16, kind="ExternalInput")
    t2 = nc.dram_tensor("t2", [VPAD, C], BF16, kind="ExternalInput")
    blockP = nc.dram_tensor("blockP", [128, 128], BF16, kind="ExternalInput")
    blockPT = nc.dram_tensor("blockPT", [128, 128], BF16, kind="ExternalInput")
    bcast8 = nc.dram_tensor("bcast8", [BL, 128], BF16, kind="ExternalInput")
    iota_rep = nc.dram_tensor("iota_rep", [128, CHW], BF16, kind="ExternalInput")
    sadj = nc.dram_tensor("sadj", [128, 1], BF16, kind="ExternalInput")

    r_out = nc.dram_tensor("r_out", [128, S], BF16, kind="ExternalOutput")
    d_out = nc.dram_tensor("d_out", [128, S], BF16, kind="ExternalOutput")
    num_out = nc.dram_tensor("num_out", [1, 2 * NCHUNK], F32, kind="ExternalOutput")

    with ExitStack() as ctx:
        tc = ctx.enter_context(tile.TileContext(nc))
        singles = ctx.enter_context(tc.tile_pool(name="singles", bufs=1))
        big = ctx.enter_context(tc.tile_pool(name="big", bufs=1))
        scratch = ctx.enter_context(tc.tile_pool(name="scratch", bufs=3))
        psum = ctx.enter_context(tc.tile_pool(name="psum", bufs=2, space="PSUM"))
        psum2 = ctx.enter_context(tc.tile_pool(name="psum2", bufs=2, space="PSUM"))

        # input DMAs spread across queues for parallel issue
        xt_sb = singles.tile([128, T // 128 * BL], I32)
        nc.sync.dma_start(out=xt_sb[:], in_=x_t[:])
        tagsf_sb = singles.tile([BL, T], BF16)
        nc.scalar.dma_start(out=tagsf_sb[:], in_=tags_f[:])
        bcast8_sb = singles.tile([BL, 128], BF16)
        nc.scalar.dma_start(out=bcast8_sb[:], in_=bcast8[:])
        iotar_sb = singles.tile([128, CHW], BF16)
        nc.scalar.dma_start(out=iotar_sb[:], in_=iota_rep[:])
        blockP_sb = singles.tile([128, 128], BF16)
        nc.scalar.dma_start(out=blockP_sb[:], in_=blockP[:])
        blockPT_sb = singles.tile([128, 128], BF16)
        nc.scalar.dma_start(out=blockPT_sb[:], in_=blockPT[:])
        sadj_sb = singles.tile([128, 1], BF16)
        nc.scalar.dma_start(out=sadj_sb[:], in_=sadj[:])

        TM = big.tile([128, T], BF16)
        G = big.tile([128, T], BF16)
        EXPG = big.tile([128, T], BF16)
        W = big.tile([128, T], BF16)
        num_sb = singles.tile([1, 2 * NCHUNK], F32)

        nc.vector.memset(num_sb[:], 0.0)

        EXPGap = EXPG[:]
        Gap = G[:]

        ones_sb = singles.tile([128, 1], BF16)
        nc.vector.memset(ones_sb[:], 1.0)
        r_sb = big.tile([128, S], BF16)
        nc.vector.memset(r_sb[:], 1.0)
        d_sb = big.tile([128, S], BF16)

        # Host permutes the token order so that G comes out SEGMENT-MAJOR:
        # G column k*S + s = token s*L + k. Chunk c of the gather therefore
        # holds exactly scan step k=c's emission block, so the forward scan
        # chases the gather; with the balanced gather order below the
        # backward scan chases from the other end.
        GORDER = [0, 7, 1, 6, 2, 5, 3, 4]

        # --- numerator W build first: only needs tags, runs during gather ---
        for c in GORDER:
            c0 = c * CHW
            psA = psum.tile([128, CHW], F32, tag="ps")
            nc.tensor.matmul(psA[:], lhsT=bcast8_sb[:],
                             rhs=tagsf_sb[:, c0:c0 + CHW], start=True, stop=True)
            nc.vector.tensor_tensor(out=W[:, c0:c0 + CHW], in0=psA[:],
                                    in1=iotar_sb[:], op=mybir.AluOpType.is_equal)

        # --- gather: one merged indirect DMA per chunk (4096 descriptors).
        # Offsets enumerate (partition, col) C-order; each offset owns 16
        # contiguous bf16 of the dest view.
        ncc = CHW // 16
        for c in GORDER:
            c0 = c * CHW
            nc.gpsimd.indirect_dma_start(
                out=TM[:, c0:c0 + CHW],
                out_offset=None,
                in_=t2[:],
                in_offset=bass.IndirectOffsetOnAxis(
                    ap=xt_sb[:, c * ncc:(c + 1) * ncc], axis=0),
            )

        # --- per chunk (in gather order): XBAR block-transpose + exp ---
        # out view (128, nb, 128): out[p, b, j] = TM[j, c0 + b*128 + p].
        nbl = CHW // 128
        for i, c in enumerate(GORDER):
            c0 = c * CHW
            eng = nc.sync if i % 2 == 0 else nc.scalar
            eng.dma_start_transpose(
                out=bass.AP(tensor=Gap.tensor, offset=c0,
                            ap=[[Gap.ap[0][0], 128], [128, nbl], [1, 128]]),
                in_=TM[:, c0:c0 + CHW])
            nc.scalar.activation(EXPG[:, c0:c0 + CHW], G[:, c0:c0 + CHW],
                                 mybir.ActivationFunctionType.Exp)
            if c == 0:
                nc.vector.tensor_mul(EXPG[:, 0:1], EXPG[:, 0:1], sadj_sb[:])

        # --- numerator em-mul on Pool (idle after gather issue);
        # reduction via PE ones-matmul accumulated into one PSUM tile ---
        scr2s = {}
        for c in GORDER:
            c0 = c * CHW
            scr2 = scratch.tile([128, CHW], BF16, tag=f"scr2_{c % 3}")
            nc.gpsimd.tensor_mul(scr2[:], G[:, c0:c0 + CHW], W[:, c0:c0 + CHW])
            scr2s[c] = scr2

        # --- scans chase the gather: issue order matches chunk readiness ---
        psN = psum2.tile([1, CHW], F32, tag="psN")

        def fw(k):
            psR = psum2.tile([128, S], F32, tag="psR")
            nc.tensor.matmul(psR[:], lhsT=blockP_sb[:], rhs=r_sb[:],
                             start=True, stop=True)
            nc.vector.tensor_mul(r_sb[:], psR[:], EXPG[:, k * S:(k + 1) * S])

        def bw(k):
            psD = psum2.tile([128, S], F32, tag="psD")
            nc.tensor.matmul(psD[:], lhsT=blockPT_sb[:], rhs=d_sb[:],
                             start=True, stop=True)
            nc.vector.tensor_mul(d_sb[:], psD[:], EXPG[:, k * S:(k + 1) * S])

        def dinit():
            nc.vector.tensor_copy(d_sb[:], EXPG[:, (L - 1) * S:L * S])

        def accum(i, c):
            nc.tensor.matmul(psN[:], lhsT=ones_sb[:], rhs=scr2s[c][:],
                             start=(i == 0), stop=(i == NCHUNK - 1))

        # slot-ordered issue: op runs when its chunk lands
        fw(0); accum(0, 0)
        dinit(); accum(1, 7)
        fw(1); accum(2, 1)
        bw(6); accum(3, 6)
        fw(2); accum(4, 2)
        bw(5); accum(5, 5)
        fw(3); accum(6, 3)
        fw(4); accum(7, 4)
        bw(4)
        fw(5)
        bw(3)
        fw(6)
        bw(2)
        fw(7)
        bw(1)
        bw(0)
        nc.vector.reduce_sum(out=num_sb[:, 0:1], in_=psN[:],
                             axis=mybir.AxisListType.X)

        nc.sync.dma_start(out=r_out[:], in_=r_sb[:])
        nc.sync.dma_start(out=num_out[:], in_=num_sb[:])
        nc.sync.dma_start(out=d_out[:], in_=d_sb[:])
    return nc


def _host_prep(embedding, fc_w, fc_b, trans, start):
    P_eff64 = np.exp(trans.astype(np.float64) + fc_b[None, :].astype(np.float64))
    colsum = P_eff64.sum(0)
    start_adj = (np.exp(start.astype(np.float64) + fc_b) / colsum).astype(np.float32)
    trans_n = (trans + fc_b[None, :]).astype(np.float32)
    P_eff32 = P_eff64.astype(np.float32)

    eye8 = np.eye(BL, dtype=np.float32)
    return dict(
        P_eff=P_eff64,
        trans_n=trans_n.astype(np.float64),
        blockP=np.ascontiguousarray(np.kron(eye8, P_eff32)).astype(NPBF),
        blockPT=np.ascontiguousarray(np.kron(eye8, P_eff32.T.copy())).astype(NPBF),
        bcast8=np.ascontiguousarray(np.kron(eye8, np.ones((1, C), np.float32))).astype(NPBF),
        iota_rep=np.ascontiguousarray(
            np.tile(np.tile(np.arange(C, dtype=np.float32), BL)[:, None],
                    (1, CHW))).astype(NPBF),
        sadj=np.ascontiguousarray(np.tile(start_adj, BL)[:, None]).astype(NPBF),
    )


LAST_RESULTS = {}


def _run(nc, in_maps, label):
    res = run_bass_kernel_spmd(nc, in_maps, core_ids=list(range(NCORES)),
                               trace=_TRACE)
    if res.exec_time_ns is not None:
        LAST_EXEC_NS[label] = res.exec_time_ns
    LAST_RESULTS[label] = res
    return res.results


def kernel(x, tags, embedding, fc_w, fc_b, start_transitions, end_transitions,
           transitions):
    x = np.asarray(x, np.int32)
    tags = np.asarray(tags, np.int32)
    embedding = np.asarray(embedding, np.float32)
    fc_w = np.asarray(fc_w, np.float32)
    fc_b = np.asarray(fc_b, np.float32)
    trans = np.asarray(transitions, np.float32)
    start = np.asarray(start_transitions, np.float32)
    end = np.asarray(end_transitions, np.float32)

    prep = _host_prep(embedding, fc_w, fc_b, trans, start)

    if "t2" not in _CACHE:
        nc1 = build_t2_kernel()
        nc1.finalize()
        _CACHE["t2"] = nc1
    if "main" not in _CACHE:
        nc2 = build_main_kernel()
        nc2.finalize()
        _CACHE["main"] = nc2

    # ---- launch 1: t2 = emb_pad @ fc_w (bf16 out), vocab-sharded ----
    emb_pad_T = np.zeros((E, VPAD), np.float32)
    emb_pad_T[:, :V] = embedding.T
    in1 = [{"embT_s": np.ascontiguousarray(emb_pad_T[:, k * VSH:(k + 1) * VSH]),
            "fc_w": fc_w} for k in range(NCORES)]
    res1 = _run(_CACHE["t2"], in1, "t2")
    # t2_s comes back transposed (C, VSH)
    t2_full = np.concatenate(
        [np.asarray(res1[k]["t2_s"]).T for k in range(NCORES)], axis=0)
    t2_full = np.ascontiguousarray(t2_full)          # (VPAD, C) bf16

    # ---- launch 2: main kernel, batch-sharded ----
    # permute tokens so G comes out segment-major: position c holds token
    # sigma(c) = (c % S)*L + c//S  (inverse of t -> (t%L)*S + t//L)
    sigma = (np.arange(T) % S) * L + np.arange(T) // S
    x_perm = x[:, sigma]
    tags_m = np.where(x_perm != 0, tags[:, sigma], C).astype(NPBF)
    in2 = []
    for k in range(NCORES):
        sl = slice(k * BL, (k + 1) * BL)
        xt = x_perm[sl].reshape(BL, T // 128, 128).transpose(2, 1, 0) \
                       .reshape(128, T // 128 * BL)
        in2.append({
            "x_t": np.ascontiguousarray(xt),
            "tags_f": np.ascontiguousarray(tags_m[sl]),
            "t2": t2_full,
            "blockP": prep["blockP"], "blockPT": prep["blockPT"],
            "bcast8": prep["bcast8"],
            "iota_rep": prep["iota_rep"], "sadj": prep["sadj"],
        })
    res2 = _run(_CACHE["main"], in2, "main")

    # ---- host combine (float64, vectorized) ----
    lengths = (x != 0).sum(1)                        # (B,)
    start64 = start.astype(np.float64)
    end64 = end.astype(np.float64)
    fcb64 = fc_b.astype(np.float64)
    Pe = prep["P_eff"]                               # (C, C) float64
    t264 = t2_full.astype(np.float64)                # (VPAD, C)
    exp_end = np.exp(end64)

    em_total = sum(float(np.asarray(res2[k]["num_out"], np.float64).sum())
                   for k in range(NCORES))
    r = np.concatenate(
        [np.asarray(res2[k]["r_out"]).astype(np.float64).reshape(BL, C, S)
         for k in range(NCORES)], axis=0)            # (B, C, S)
    d = np.concatenate(
        [np.asarray(res2[k]["d_out"]).astype(np.float64).reshape(BL, C, S)
         for k in range(NCORES)], axis=0)            # (B, C, S)

    num = start64[tags[:, 0]] + fcb64[tags[:, 0]]
    num += end64[tags[np.arange(B), lengths - 1]]
    # transition terms (pure tags/params, no device data)
    maskf = (x[:, 1:] != 0).astype(np.float64)
    num += (prep["trans_n"][tags[:, :-1], tags[:, 1:]] * maskf).sum(axis=1)

    # full-segment junction chain: for s in 1..sstar-1:
    #   logZ += log(r[:,:,s-1] @ (Pe @ d[:,:,s])) - log(r[:,:,s].sum())
    sstar = (lengths - 1) // L                       # (B,)
    cs = np.einsum('cd,bds->bcs', Pe, d)             # (B, C, S)
    t1 = np.einsum('bcs,bcs->bs', r[:, :, :-1], cs[:, :, 1:])   # junction at s=1..S-1
    rs = r.sum(axis=1)                               # (B, S)
    s_idx = np.arange(1, S)[None, :]                 # (1, S-1)
    jmask = s_idx <= (sstar[:, None] - 1)            # (B, S-1)
    terms = np.where(jmask, np.log(t1) - np.log(rs[:, 1:]), 0.0)
    logZ = terms.sum(axis=1)                         # (B,)

    # ragged tail: exact alpha recursion from segment sstar-1's r
    alpha = r[np.arange(B), :, sstar - 1].copy()     # (B, C)
    tail_len = lengths - sstar * L                   # in [1, L]
    for t_off in range(L):
        active = t_off < tail_len                    # (B,)
        t_idx = np.minimum(sstar * L + t_off, T - 1)
        w = np.exp(t264[x[np.arange(B), t_idx]] + fcb64[None, :])   # (B, C)
        alpha_new = (alpha @ Pe) * w
        alpha = np.where(active[:, None], alpha_new, alpha)
    logZ += np.log(alpha @ exp_end)

    total = -(num - logZ).sum() - em_total
    return np.array(total, dtype=np.float32)


# revision 35
# speedup vs baseline: 1.1441x; 1.0047x over previous
"""CRF negative-log-likelihood kernel for Trainium2 (8 NeuronCores, batch-sharded).

Algorithm:
  - Launch 1 (vocab-sharded): t2 = embedding @ fc_w in bf16. Host pre-transposes
    the embedding shard so the kernel is just convert-to-bf16 + 50 matmuls
    (lhsT = embT chunk, rhs = fc_w), no PE transposes. Output t2 is bf16
    (32B rows) to halve gather traffic.
  - Launch 2 (batch-sharded, 8 rows/core, bf16 compute): merged indirect-DMA
    gathers of t2 rows (8 calls, 4096 descriptors each), bf16 PE-block
    transposes into class-on-partition layout, numerator via one-hot matmul +
    multiply-reduce, and a segmented forward/backward scan (L=16 steps, S=256
    segments on the free dim) in linear space with the two scan chains
    interleaved so vector muls hide behind the other chain's matmuls.
  - Host (float64, vectorized): rank-1 junction chain across segments, exact
    partial segment for each row's ragged tail, final scalar assembly.
"""
import sys
sys.path.insert(0, "/opt/trn_rl_repo")
import numpy as np
import ml_dtypes
from contextlib import ExitStack

import concourse.bass as bass
import concourse.bacc as bacc_mod
import concourse.mybir as mybir
import concourse.tile as tile
from concourse.masks import make_identity
from concourse.bass_utils import run_bass_kernel_spmd

F32 = mybir.dt.float32
BF16 = mybir.dt.bfloat16
I32 = mybir.dt.int32
NPBF = ml_dtypes.bfloat16

V, E, C = 50257, 128, 16
B, T = 64, 4096
L, S = 8, 512
VPAD = 51200
VSH = VPAD // 8
BL = 8
NCHUNK = 8
CHW = T // NCHUNK
NCORES = 8

LAST_EXEC_NS = {}
_TRACE = False
_CACHE = {}


def build_t2_kernel():
    nc = bacc_mod.Bacc()
    # embT_s: host-pretransposed shard, (E, VSH) f32
    embT_s = nc.dram_tensor("embT_s", [E, VSH], F32, kind="ExternalInput")
    fc_w = nc.dram_tensor("fc_w", [E, C], F32, kind="ExternalInput")
    # t2 shard TRANSPOSED: (C, VSH); host un-transposes. One stationary fcw,
    # wide matmuls (out 16 x 640), contiguous out-DMA.
    t2_s = nc.dram_tensor("t2_s", [C, VSH], BF16, kind="ExternalOutput")

    NGRP = 10                   # DMA/convert/matmul granularity
    GW = VSH // NGRP            # 640 columns per group
    with ExitStack() as ctx:
        tc = ctx.enter_context(tile.TileContext(nc))
        singles = ctx.enter_context(tc.tile_pool(name="singles", bufs=1))
        psum = ctx.enter_context(tc.tile_pool(name="psum", bufs=4, space="PSUM"))

        fcw_f32 = singles.tile([E, C], F32)
        nc.scalar.dma_start(out=fcw_f32[:], in_=fc_w[:])
        fcw_bf = singles.tile([E, C], BF16)
        nc.vector.tensor_copy(fcw_bf[:], fcw_f32[:])

        EMBT = singles.tile([128, VSH], F32)
        EMBTb = singles.tile([128, VSH], BF16)
        T2T = singles.tile([C, VSH], BF16)
        for g in range(NGRP):
            eng = nc.sync if g % 2 == 0 else nc.scalar
            eng.dma_start(out=EMBT[:, g * GW:(g + 1) * GW],
                          in_=embT_s[:, g * GW:(g + 1) * GW])
            if g % 2 == 0:
                nc.vector.tensor_copy(EMBTb[:, g * GW:(g + 1) * GW],
                                      EMBT[:, g * GW:(g + 1) * GW])
            else:
                nc.scalar.copy(EMBTb[:, g * GW:(g + 1) * GW],
                               EMBT[:, g * GW:(g + 1) * GW])
        MW = GW // 2            # 320 f32 fits a 2KB PSUM bank
        for m in range(2 * NGRP):
            ps2 = psum.tile([C, MW], F32, tag="p2")
            nc.tensor.matmul(ps2[:], lhsT=fcw_bf[:],
                             rhs=EMBTb[:, m * MW:(m + 1) * MW],
                             start=True, stop=True)
            if m % 2 == 0:
                nc.vector.tensor_copy(T2T[:, m * MW:(m + 1) * MW], ps2[:])
            else:
                nc.scalar.copy(T2T[:, m * MW:(m + 1) * MW], ps2[:])
        nc.sync.dma_start(out=t2_s[:], in_=T2T[:])
    return nc


def _strided(base_ap, k, step, count):
    return bass.AP(tensor=base_ap.tensor, offset=base_ap.offset + k,
                   ap=[base_ap.ap[0], [step, count]])


def build_main_kernel():
    nc = bacc_mod.Bacc()
    x_t = nc.dram_tensor("x_t", [128, T // 128 * BL], I32, kind="ExternalInput")
    tags_f = nc.dram_tensor("tags_f", [BL, T], BF16, kind="ExternalInput")
    t2 = nc.dram_tensor("t2", [VPAD, C], BF16, kind="ExternalInput")
    blockP = nc.dram_tensor("blockP", [128, 128], BF16, kind="ExternalInput")
    blockPT = nc.dram_tensor("blockPT", [128, 128], BF16, kind="ExternalInput")
    bcast8 = nc.dram_tensor("bcast8", [BL, 128], BF16, kind="ExternalInput")
    iota_rep = nc.dram_tensor("iota_rep", [128, CHW], BF16, kind="ExternalInput")
    sadj = nc.dram_tensor("sadj", [128, 1], BF16, kind="ExternalInput")

    r_out = nc.dram_tensor("r_out", [128, S], BF16, kind="ExternalOutput")
    d_out = nc.dram_tensor("d_out", [128, S], BF16, kind="ExternalOutput")
    num_out = nc.dram_tensor("num_out", [1, 2 * NCHUNK], F32, kind="ExternalOutput")

    with ExitStack() as ctx:
        tc = ctx.enter_context(tile.TileContext(nc))
        singles = ctx.enter_context(tc.tile_pool(name="singles", bufs=1))
        big = ctx.enter_context(tc.tile_pool(name="big", bufs=1))
        scratch = ctx.enter_context(tc.tile_pool(name="scratch", bufs=3))
        psum = ctx.enter_context(tc.tile_pool(name="psum", bufs=2, space="PSUM"))
        psum2 = ctx.enter_context(tc.tile_pool(name="psum2", bufs=2, space="PSUM"))

        # input DMAs spread across queues for parallel issue
        xt_sb = singles.tile([128, T // 128 * BL], I32)
        nc.sync.dma_start(out=xt_sb[:], in_=x_t[:])
        tagsf_sb = singles.tile([BL, T], BF16)
        nc.scalar.dma_start(out=tagsf_sb[:], in_=tags_f[:])
        bcast8_sb = singles.tile([BL, 128], BF16)
        nc.scalar.dma_start(out=bcast8_sb[:], in_=bcast8[:])
        iotar_sb = singles.tile([128, CHW], BF16)
        nc.scalar.dma_start(out=iotar_sb[:], in_=iota_rep[:])
        blockP_sb = singles.tile([128, 128], BF16)
        nc.scalar.dma_start(out=blockP_sb[:], in_=blockP[:])
        blockPT_sb = singles.tile([128, 128], BF16)
        nc.scalar.dma_start(out=blockPT_sb[:], in_=blockPT[:])
        sadj_sb = singles.tile([128, 1], BF16)
        nc.scalar.dma_start(out=sadj_sb[:], in_=sadj[:])

        TM = big.tile([128, T], BF16)
        G = big.tile([128, T], BF16)
        EXPG = big.tile([128, T], BF16)
        W = big.tile([128, T], BF16)
        num_sb = singles.tile([1, 2 * NCHUNK], F32)

        nc.vector.memset(num_sb[:], 0.0)

        EXPGap = EXPG[:]
        Gap = G[:]

        ones_sb = singles.tile([128, 1], BF16)
        nc.vector.memset(ones_sb[:], 1.0)
        r_sb = big.tile([128, S], BF16)
        nc.vector.memset(r_sb[:], 1.0)
        d_sb = big.tile([128, S], BF16)

        # Host permutes the token order so that G comes out SEGMENT-MAJOR:
        # G column k*S + s = token s*L + k. Chunk c of the gather therefore
        # holds exactly scan step k=c's emission block, so the forward scan
        # chases the gather; with the balanced gather order below the
        # backward scan chases from the other end.
        GORDER = [0, 7, 1, 6, 2, 5, 3, 4]

        # --- numerator W build first: only needs tags, runs during gather ---
        for c in GORDER:
            c0 = c * CHW
            psA = psum.tile([128, CHW], F32, tag="ps")
            nc.tensor.matmul(psA[:], lhsT=bcast8_sb[:],
                             rhs=tagsf_sb[:, c0:c0 + CHW], start=True, stop=True)
            nc.vector.tensor_tensor(out=W[:, c0:c0 + CHW], in0=psA[:],
                                    in1=iotar_sb[:], op=mybir.AluOpType.is_equal)

        # --- gather: one merged indirect DMA per chunk (4096 descriptors).
        # Offsets enumerate (partition, col) C-order; each offset owns 16
        # contiguous bf16 of the dest view.
        ncc = CHW // 16
        for c in GORDER:
            c0 = c * CHW
            nc.gpsimd.indirect_dma_start(
                out=TM[:, c0:c0 + CHW],
                out_offset=None,
                in_=t2[:],
                in_offset=bass.IndirectOffsetOnAxis(
                    ap=xt_sb[:, c * ncc:(c + 1) * ncc], axis=0),
            )

        # --- per chunk (in gather order): XBAR block-transpose + exp ---
        # out view (128, nb, 128): out[p, b, j] = TM[j, c0 + b*128 + p].
        nbl = CHW // 128
        for i, c in enumerate(GORDER):
            c0 = c * CHW
            eng = nc.sync
            eng.dma_start_transpose(
                out=bass.AP(tensor=Gap.tensor, offset=c0,
                            ap=[[Gap.ap[0][0], 128], [128, nbl], [1, 128]]),
                in_=TM[:, c0:c0 + CHW])
            nc.scalar.activation(EXPG[:, c0:c0 + CHW], G[:, c0:c0 + CHW],
                                 mybir.ActivationFunctionType.Exp)
            if c == 0:
                nc.vector.tensor_mul(EXPG[:, 0:1], EXPG[:, 0:1], sadj_sb[:])

        # --- numerator em-mul on Pool (idle after gather issue);
        # reduction via PE ones-matmul accumulated into one PSUM tile ---
        scr2s = {}
        for c in GORDER:
            c0 = c * CHW
            scr2 = scratch.tile([128, CHW], BF16, tag=f"scr2_{c % 3}")
            nc.gpsimd.tensor_mul(scr2[:], G[:, c0:c0 + CHW], W[:, c0:c0 + CHW])
            scr2s[c] = scr2

        # --- scans chase the gather: issue order matches chunk readiness ---
        psN = psum2.tile([1, CHW], F32, tag="psN")

        def fw(k):
            psR = psum2.tile([128, S], F32, tag="psR")
            nc.tensor.matmul(psR[:], lhsT=blockP_sb[:], rhs=r_sb[:],
                             start=True, stop=True)
            nc.vector.tensor_mul(r_sb[:], psR[:], EXPG[:, k * S:(k + 1) * S])

        def bw(k):
            psD = psum2.tile([128, S], F32, tag="psD")
            nc.tensor.matmul(psD[:], lhsT=blockPT_sb[:], rhs=d_sb[:],
                             start=True, stop=True)
            nc.vector.tensor_mul(d_sb[:], psD[:], EXPG[:, k * S:(k + 1) * S])

        def dinit():
            nc.vector.tensor_copy(d_sb[:], EXPG[:, (L - 1) * S:L * S])

        def accum(i, c):
            nc.tensor.matmul(psN[:], lhsT=ones_sb[:], rhs=scr2s[c][:],
                             start=(i == 0), stop=(i == NCHUNK - 1))

        # slot-ordered issue: op runs when its chunk lands
        fw(0); accum(0, 0)
        dinit(); accum(1, 7)
        fw(1); accum(2, 1)
        bw(6); accum(3, 6)
        fw(2); accum(4, 2)
        bw(5); accum(5, 5)
        fw(3); accum(6, 3)
        fw(4); accum(7, 4)
        bw(4)
        fw(5)
        bw(3)
        fw(6)
        bw(2)
        fw(7)
        bw(1)
        bw(0)
        nc.vector.reduce_sum(out=num_sb[:, 0:1], in_=psN[:],
                             axis=mybir.AxisListType.X)

        nc.sync.dma_start(out=r_out[:], in_=r_sb[:])
        nc.sync.dma_start(out=num_out[:], in_=num_sb[:])
        nc.sync.dma_start(out=d_out[:], in_=d_sb[:])
    return nc


def _host_prep(embedding, fc_w, fc_b, trans, start):
    P_eff64 = np.exp(trans.astype(np.float64) + fc_b[None, :].astype(np.float64))
    colsum = P_eff64.sum(0)
    start_adj = (np.exp(start.astype(np.float64) + fc_b) / colsum).astype(np.float32)
    trans_n = (trans + fc_b[None, :]).astype(np.float32)
    P_eff32 = P_eff64.astype(np.float32)

    eye8 = np.eye(BL, dtype=np.float32)
    return dict(
        P_eff=P_eff64,
        trans_n=trans_n.astype(np.float64),
        blockP=np.ascontiguousarray(np.kron(eye8, P_eff32)).astype(NPBF),
        blockPT=np.ascontiguousarray(np.kron(eye8, P_eff32.T.copy())).astype(NPBF),
        bcast8=np.ascontiguousarray(np.kron(eye8, np.ones((1, C), np.float32))).astype(NPBF),
        iota_rep=np.ascontiguousarray(
            np.tile(np.tile(np.arange(C, dtype=np.float32), BL)[:, None],
                    (1, CHW))).astype(NPBF),
        sadj=np.ascontiguousarray(np.tile(start_adj, BL)[:, None]).astype(NPBF),
    )


LAST_RESULTS = {}


def _run(nc, in_maps, label):
    res = run_bass_kernel_spmd(nc, in_maps, core_ids=list(range(NCORES)),
                               trace=_TRACE)
    if res.exec_time_ns is not None:
        LAST_EXEC_NS[label] = res.exec_time_ns
    LAST_RESULTS[label] = res
    return res.results


def kernel(x, tags, embedding, fc_w, fc_b, start_transitions, end_transitions,
           transitions):
    x = np.asarray(x, np.int32)
    tags = np.asarray(tags, np.int32)
    embedding = np.asarray(embedding, np.float32)
    fc_w = np.asarray(fc_w, np.float32)
    fc_b = np.asarray(fc_b, np.float32)
    trans = np.asarray(transitions, np.float32)
    start = np.asarray(start_transitions, np.float32)
    end = np.asarray(end_transitions, np.float32)

    prep = _host_prep(embedding, fc_w, fc_b, trans, start)

    if "t2" not in _CACHE:
        nc1 = build_t2_kernel()
        nc1.finalize()
        _CACHE["t2"] = nc1
    if "main" not in _CACHE:
        nc2 = build_main_kernel()
        nc2.finalize()
        _CACHE["main"] = nc2

    # ---- launch 1: t2 = emb_pad @ fc_w (bf16 out), vocab-sharded ----
    emb_pad_T = np.zeros((E, VPAD), np.float32)
    emb_pad_T[:, :V] = embedding.T
    in1 = [{"embT_s": np.ascontiguousarray(emb_pad_T[:, k * VSH:(k + 1) * VSH]),
            "fc_w": fc_w} for k in range(NCORES)]
    res1 = _run(_CACHE["t2"], in1, "t2")
    # t2_s comes back transposed (C, VSH)
    t2_full = np.concatenate(
        [np.asarray(res1[k]["t2_s"]).T for k in range(NCORES)], axis=0)
    t2_full = np.ascontiguousarray(t2_full)          # (VPAD, C) bf16

    # ---- launch 2: main kernel, batch-sharded ----
    # permute tokens so G comes out segment-major: position c holds token
    # sigma(c) = (c % S)*L + c//S  (inverse of t -> (t%L)*S + t//L)
    sigma = (np.arange(T) % S) * L + np.arange(T) // S
    x_perm = x[:, sigma]
    tags_m = np.where(x_perm != 0, tags[:, sigma], C).astype(NPBF)
    in2 = []
    for k in range(NCORES):
        sl = slice(k * BL, (k + 1) * BL)
        xt = x_perm[sl].reshape(BL, T // 128, 128).transpose(2, 1, 0) \
                       .reshape(128, T // 128 * BL)
        in2.append({
            "x_t": np.ascontiguousarray(xt),
            "tags_f": np.ascontiguousarray(tags_m[sl]),
            "t2": t2_full,
            "blockP": prep["blockP"], "blockPT": prep["blockPT"],
            "bcast8": prep["bcast8"],
            "iota_rep": prep["iota_rep"], "sadj": prep["sadj"],
        })
    res2 = _run(_CACHE["main"], in2, "main")

    # ---- host combine (float64, vectorized) ----
    lengths = (x != 0).sum(1)                        # (B,)
    start64 = start.astype(np.float64)
    end64 = end.astype(np.float64)
    fcb64 = fc_b.astype(np.float64)
    Pe = prep["P_eff"]                               # (C, C) float64
    t264 = t2_full.astype(np.float64)                # (VPAD, C)
    exp_end = np.exp(end64)

    em_total = sum(float(np.asarray(res2[k]["num_out"], np.float64).sum())
                   for k in range(NCORES))
    r = np.concatenate(
        [np.asarray(res2[k]["r_out"]).astype(np.float64).reshape(BL, C, S)
         for k in range(NCORES)], axis=0)            # (B, C, S)
    d = np.concatenate(
        [np.asarray(res2[k]["d_out"]).astype(np.float64).reshape(BL, C, S)
         for k in range(NCORES)], axis=0)            # (B, C, S)

    num = start64[tags[:, 0]] + fcb64[tags[:, 0]]
    num += end64[tags[np.arange(B), lengths - 1]]
    # transition terms (pure tags/params, no device data)
    maskf = (x[:, 1:] != 0).astype(np.float64)
    num += (prep["trans_n"][tags[:, :-1], tags[:, 1:]] * maskf).sum(axis=1)

    # full-segment junction chain: for s in 1..sstar-1:
    #   logZ += log(r[:,:,s-1] @ (Pe @ d[:,:,s])) - log(r[:,:,s].sum())
    sstar = (lengths - 1) // L                       # (B,)
    cs = np.einsum('cd,bds->bcs', Pe, d)             # (B, C, S)
    t1 = np.einsum('bcs,bcs->bs', r[:, :, :-1], cs[:, :, 1:])   # junction at s=1..S-1
    rs = r.sum(axis=1)                               # (B, S)
    s_idx = np.arange(1, S)[None, :]                 # (1, S-1)
    jmask = s_idx <= (sstar[:, None] - 1)            # (B, S-1)
    terms = np.where(jmask, np.log(t1) - np.log(rs[:, 1:]), 0.0)
    logZ = terms.sum(axis=1)                         # (B,)

    # ragged tail: exact alpha recursion from segment sstar-1's r
    alpha = r[np.arange(B), :, sstar - 1].copy()     # (B, C)
    tail_len = lengths - sstar * L                   # in [1, L]
    for t_off in range(L):
        active = t_off < tail_len                    # (B,)
        t_idx = np.minimum(sstar * L + t_off, T - 1)
        w = np.exp(t264[x[np.arange(B), t_idx]] + fcb64[None, :])   # (B, C)
        alpha_new = (alpha @ Pe) * w
        alpha = np.where(active[:, None], alpha_new, alpha)
    logZ += np.log(alpha @ exp_end)

    total = -(num - logZ).sum() - em_total
    return np.array(total, dtype=np.float32)
